# revision 35
# baseline (speedup 1.0000x reference)
"""Causal self-attention Trainium2 kernel (B=8, T=1024, C=768, H=12 heads).

Strategy: data-parallel over batch — one batch element per NeuronCore (8 cores).
Per core, everything is computed in a "transposed" layout so that no on-device
transposes are needed:

  qT, kT  [C, T]   = w_attn_{q,k}.T @ x.T          (x.T supplied by host)
  v_aug   [T, 780] = x @ [w_attn_v | 0]  (+ ones column per head, stride 65)
  sT_h    [Tk, Tq] = kT_h.T-slices @ qT_h          (keys on partitions, the two
                                                    heads of a pair run as
                                                    concurrent row-tiled MMs)
  eT      = exp(sT / 8), fp16, causal mask via one batched 2-head multiply
  yT_aug  [65, Tq] = v_aug_h.T @ eT                (row 64 = softmax row-sums)
  yT_norm = yT * broadcast(1/sums)                 (broadcast via gpsimd
                                                    partition_broadcast)
  out     [T, C]   = yT_norm.T-slices @ w_proj

All matmul operands are fp16 (1 col/cycle PE rate, fast weight loads, half the
DMA traffic of fp32); PSUM accumulation stays fp32, final output is fp32.

The issue order forms a software pipeline tuned so no engine starves: the
scalar-engine exp latency ((N+352)/1.2 ns + ~0.3us semaphore hops) is hidden
by weaving ~0.5us filler matmul chunks (QK / v / projection, expressed as
Python generators) between every score and attv step.  Phase alpha covers
query half 0 (+ all QK and most v tiles); phase beta covers query half 1 with
the output projection as filler, split by contraction (heads 0-2 -> fp16 SBUF
partial, 3-4 as late filler, 5 in the tail) so the post-attention tail stays
short.  Per-head-pair normalization is deferred by one block so the in-order
PE queue never waits on the sums chain.  Head DMAs are split between the
sync and scalar queues (~0.65us serial issue cost per descriptor per queue);
yT staging DMAs and the sum broadcasts ride the otherwise-idle gpsimd queue.
"""
import sys

sys.path.insert(0, "/opt/trn_rl_repo")

import numpy as np

import concourse.bass as bass
import concourse.bacc as bacc
import concourse.tile as tile
import concourse.mybir as mybir
from concourse.bass_utils import run_bass_kernel_spmd

f32 = mybir.dt.float32
fp16 = mybir.dt.float16
EXP = mybir.ActivationFunctionType.Exp

B, T, C = 8, 1024, 768
H, D = 12, 64
DA = D + 1        # per-head block in v: [v_h(64) | 1]
HB = 2 * DA       # head-pair stride
VW = H * DA       # 780
NK = C // 128     # 6 contraction tiles
NT = T // 128     # 8 token tiles
SCALE = 1.0 / np.sqrt(D)


def build():
    nc = bacc.Bacc("TRN2", target_bir_lowering=False, debug=False)
    xT = nc.dram_tensor("xT", [C, T], fp16, kind="ExternalInput")
    wq = nc.dram_tensor("wq", [2 * NK, 128, NK, 128], fp16, kind="ExternalInput")
    wv = nc.dram_tensor("wv", [NK, 128, VW], fp16, kind="ExternalInput")
    wp = nc.dram_tensor("wp", [NK, 128, C], fp16, kind="ExternalInput")
    msk = nc.dram_tensor("msk", [128, 256], fp16, kind="ExternalInput")
    onesc = nc.dram_tensor("onesc", [128, H], fp16, kind="ExternalInput")
    out = nc.dram_tensor("out", [T, C], f32, kind="ExternalOutput")

    with tile.TileContext(nc) as tc:
        with (
            tc.tile_pool(name="const", bufs=1) as const,
            tc.tile_pool(name="wqp", bufs=4) as wqp,
            tc.tile_pool(name="exp", bufs=4) as expp,
            tc.tile_pool(name="psc", bufs=3, space="PSUM") as psc,
            tc.tile_pool(name="psm", bufs=2, space="PSUM") as psm,
        ):
            # ---- resident SBUF tensors ----
            xTall = const.tile([128, NK * T], fp16, tag="xTall")
            xT_t = [xTall[:, i * T:(i + 1) * T] for i in range(NK)]
            wvall = const.tile([128, NK * VW], fp16, tag="wvall")
            wv_t = [wvall[:, i * VW:(i + 1) * VW] for i in range(NK)]
            wvd = wvall.rearrange("p (i n) -> p i n", i=NK)
            wpall = const.tile([128, NK * C], fp16, tag="wpall")
            wp_t = [wpall[:, i * C:(i + 1) * C] for i in range(NK)]
            qkT_t = [const.tile([128, T], fp16, name=f"qks{m}", tag=f"qk{m}") for m in range(2 * NK)]
            v_t = [const.tile([128, VW], fp16, name=f"vs{t}", tag=f"v{t}") for t in range(NT)]
            yT_t = [const.tile([128, T], fp16, name=f"yTs{i}", tag=f"yT{i}") for i in range(NK)]
            part = [const.tile([128, C], fp16, name=f"prt{t}", tag=f"prt{t}") for t in range(NT)]
            msk_t = const.tile([128, 256], fp16, tag="msk")
            ones_t = const.tile([128, H], fp16, tag="ones")
            # softmax sums sit on PSUM row 64 of the attv output; a tiny
            # gpsimd DMA drops them onto partition 0 of s64r (head A at cols
            # 0:512, head B 512:1024).  They are then broadcast to all 128
            # partitions by two gpsimd partition_broadcast ops (partition-0
            # in / partition-0 out — the only HW-supported form), and the DVE
            # reciprocal + multiply normalize the staged y tiles in place
            # before the DMA into yT.
            s64v = const.tile([65, 1024], f32, tag="s64v")
            s64r = const.tile([1, 1024], f32, tag="s64r")
            onr = const.tile([65, 256], f32, tag="onr")

            xTd = xT.rearrange("(i p) n -> p i n", p=128)
            wvs = wv.rearrange("i p n -> p i n")

            wq_tiles = {}

            def wq_fetch(m, eng=None, split=False):
                wq_tiles[m] = wqp.tile([128, NK, 128], fp16, tag="wq", name=f"wq{m}")
                if split:  # halves: first matmuls start before the tail lands
                    (eng or nc.sync).dma_start(
                        out=wq_tiles[m][:, 0:3, :], in_=wq[m, :, 0:3, :]
                    )
                    (eng or nc.sync).dma_start(
                        out=wq_tiles[m][:, 3:NK, :], in_=wq[m, :, 3:NK, :]
                    )
                else:
                    (eng or nc.sync).dma_start(out=wq_tiles[m], in_=wq[m, :, :, :])

            # ---------------- building blocks (filler jobs are generators;
            # each `yield` is a ~0.5us chunk boundary for the weave) ---------
            def gen_qk(m):
                wq_t = wq_tiles[m]
                ps = psc.tile([128, 1024], f32, tag="ps", name=f"psqk{m}")

                def mm(qc, kk):
                    nc.tensor.matmul(
                        ps[:, qc * 512:(qc + 1) * 512],
                        wq_t[:, kk, :],
                        xT_t[kk][:, qc * 512:(qc + 1) * 512],
                        start=(kk == 0),
                        stop=(kk == NK - 1),
                    )

                def cp(half):
                    dst = qkT_t[m][:, half * 512:(half + 1) * 512]
                    src = ps[:, half * 512:(half + 1) * 512]
                    if m % 2 == 0:
                        nc.scalar.copy(dst, src)
                    else:
                        nc.vector.tensor_copy(dst, src)

                for kk in range(4):
                    mm(0, kk)
                yield
                for kk in range(4, NK):
                    mm(0, kk)
                cp(0)  # first half drains while the second accumulates
                for kk in range(2):
                    mm(1, kk)
                yield
                for kk in range(2, NK):
                    mm(1, kk)
                wq_tiles.pop(m)
                cp(1)

            def gen_v(t):
                ps = psc.tile([128, 1024], f32, tag="ps", name=f"psv{t}")
                for ci, kks in enumerate(((0, 1), (2, 3), (4, 5))):
                    for kk in kks:
                        for n0, nw in ((0, 512), (512, VW - 512)):
                            nc.tensor.matmul(
                                ps[:, n0:n0 + nw],
                                xT_t[kk][:, t * 128:(t + 1) * 128],
                                wv_t[kk][:, n0:n0 + nw],
                                start=(kk == 0),
                                stop=(kk == NK - 1),
                            )
                    if ci < 2:
                        yield
                if t % 2 == 0:
                    nc.scalar.copy(v_t[t], ps[:, :VW])
                else:
                    nc.vector.tensor_copy(v_t[t], ps[:, :VW])
                ones_ap = v_t[t].rearrange("p (h e) -> p h e", e=DA)[:, :, D]
                nc.vector.tensor_copy(ones_ap, ones_t)

            def gen_P(t, kk0, kk1, mode, eng=None):
                """Projection tile t over contraction tiles [kk0, kk1).
                mode: 'part' -> write fp16 partial; 'acc' -> add into partial;
                'out' -> add partial + DMA the finished row block out (on
                engine `eng`, default sync)."""
                pp = psc.tile([128, 1024], f32, tag="ps", name=f"pp{t}_{kk0}")
                for kk in range(kk0, kk1):
                    nc.tensor.matmul(
                        pp[:, 0:512],
                        yT_t[kk][:, t * 128:(t + 1) * 128],
                        wp_t[kk][:, 0:512],
                        start=(kk == kk0),
                        stop=(kk == kk1 - 1),
                    )
                yield
                for kk in range(kk0, kk1):
                    nc.tensor.matmul(
                        pp[:, 512:768],
                        yT_t[kk][:, t * 128:(t + 1) * 128],
                        wp_t[kk][:, 512:768],
                        start=(kk == kk0),
                        stop=(kk == kk1 - 1),
                    )
                if mode == "part":
                    nc.vector.tensor_copy(part[t], pp[:, :C])
                elif mode == "acc":
                    nc.vector.tensor_add(part[t], pp[:, :C], part[t])
                else:
                    ostage = expp.tile([128, C], f32, tag="ostage", bufs=4, name="ostage")
                    for h0, h1 in ((0, 384), (384, C)):
                        nc.vector.tensor_add(
                            ostage[:, h0:h1], pp[:, h0:h1], part[t][:, h0:h1]
                        )
                        (eng or nc.sync).dma_start(
                            out=out[t * 128:(t + 1) * 128, h0:h1], in_=ostage[:, h0:h1]
                        )

            def gen_noop(n):
                for _ in range(n - 1):
                    yield

            def drain(g):
                for _ in g:
                    pass

            yps = {}
            exs = {}

            def S(hp, qc, kt):
                """Scores pair (row-tiled, concurrent) + exp (+ causal mask)."""
                qT = qkT_t[hp]
                kT = qkT_t[NK + hp]
                ks = slice(kt * 128, (kt + 1) * 128)
                pos = max(kt * 128 - qc * 512, 0)
                qv = slice(qc * 512 + pos, (qc + 1) * 512)
                sp = psc.tile([128, 1024], f32, tag="ps", name="sp")
                nc.tensor.matmul(
                    sp[:, pos:512], kT[0:64, ks], qT[0:64, qv],
                    start=True, stop=True,
                )
                nc.tensor.matmul(
                    sp[:, 512 + pos:1024], kT[64:128, ks], qT[64:128, qv],
                    start=True, stop=True,
                )
                ex = expp.tile([128, 1024], fp16, tag="ex", bufs=8, name="ex")
                if pos == 0:
                    nc.scalar.activation(ex, sp, EXP, scale=float(SCALE))
                else:
                    exv = ex.rearrange("p (i n) -> p i n", i=2)[:, :, pos:512]
                    spv = sp.rearrange("p (i n) -> p i n", i=2)[:, :, pos:512]
                    nc.scalar.activation(exv, spv, EXP, scale=float(SCALE))
                if kt * 128 >= qc * 512:  # diagonal tile: mask both heads at once
                    exd = ex.rearrange("p (i n) -> p i n", i=2)[:, :, pos:pos + 128]
                    mkd = msk_t.rearrange("p (i n) -> p i n", i=2)
                    nc.vector.tensor_mul(exd, exd, mkd)
                exs[(hp, qc, kt)] = (ex, pos)

            def A(hp, qc, kt, nkt):
                """attv pair for exp tile (hp, qc, kt)."""
                if (hp, qc) not in yps:
                    yps[(hp, qc)] = (
                        psm.tile([128, 512], f32, tag="yp", name="ypA"),
                        psm.tile([128, 512], f32, tag="yp", name="ypB"),
                    )
                ypA, ypB = yps[(hp, qc)]
                ex, pos = exs.pop((hp, qc, kt))
                for yp, half in ((ypA, 0), (ypB, 1)):
                    nc.tensor.matmul(
                        yp[:DA, pos:512],
                        v_t[kt][:, hp * HB + half * DA:hp * HB + (half + 1) * DA],
                        ex[:, half * 512 + pos:(half + 1) * 512],
                        start=(kt == 0), stop=(kt == nkt - 1),
                    )

            stages = {}
            bcast = {}

            def FIN_sums(hp, qc, tail=False):
                """Issued right after the last attv of the block: pull the
                softmax sums out of PSUM row 64 (DVE copy — DMA cannot read
                PSUM), drop them onto partition 0 of s64r with one gpsimd
                SBUF-to-SBUF DMA, and broadcast them to all partitions.  This
                jumps the vector queue ahead of the block-tail casts so the
                broadcast is long done when FIN_normB consumes it."""
                with nc.allow_low_precision(reason="sums rounding is benign"):
                    for r, yp in enumerate(yps[(hp, qc)]):
                        nc.vector.tensor_copy(
                            s64v[64:65, r * 512:(r + 1) * 512], yp[D:DA, :]
                        )
                if tail:
                    return
                nc.gpsimd.dma_start(out=s64r[0:1, :], in_=s64v[64:65, :])
                bcS = expp.tile([128, 1024], f32, tag="bcS", bufs=2, name="bcS")
                nc.gpsimd.partition_broadcast(bcS, s64r[0:1, :], channels=128)
                bcast[(hp, qc)] = bcS
                # stage the unnormalized y rows right away: the copies drain
                # on the vector queue during the block-tail fills, freeing the
                # psum pair before the next block's first attv (WAR) and
                # keeping the next block's mask muls unqueued.
                st = []
                for yp in yps[(hp, qc)]:
                    stage = expp.tile([D, 512], fp16, tag="ystage", bufs=4, name="stage")
                    nc.vector.tensor_copy(stage, yp[:D, :])
                    st.append(stage)
                stages[(hp, qc)] = st
                del yps[(hp, qc)]

            def FIN_normB(hp, qc):
                """Reciprocal of the broadcast sums, normalize the staged y
                tiles in place, DMA them into yT.  Deferred to the middle of
                the next block so the in-order vector queue never waits on
                the gpsimd broadcast."""
                bcS = bcast.pop((hp, qc))
                bcR = expp.tile([128, 1024], f32, tag="bcR", bufs=2, name="bcR")
                nc.vector.reciprocal_approx_fast(bcR, bcS)
                qs = slice(qc * 512, (qc + 1) * 512)
                for r, (stage, off) in enumerate(zip(stages.pop((hp, qc)), (0, 64))):
                    nc.vector.tensor_mul(
                        stage, stage, bcR[0:D, r * 512:(r + 1) * 512]
                    )
                    # sync queue: keeps the in-order gpsimd queue free for the
                    # next block's sums broadcast (convoy breaker)
                    nc.sync.dma_start(out=yT_t[hp][off:off + 64, qs], in_=stage)

            def FIN_tail(hp, qc):
                """Fast finish for the very last block: broadcast the sums
                with two K=1 PE matmuls (the PE is idle in the tail, and this
                skips the long gpsimd broadcast chain), then reciprocal,
                normalize, ship."""
                bc = psc.tile([128, 1024], f32, tag="ps", name="bc")
                nc.tensor.matmul(
                    bc[:, 0:512], onr[64:65, 0:128], s64v[64:65, 0:512],
                    start=True, stop=True,
                )
                nc.tensor.matmul(
                    bc[:, 512:1024], onr[64:65, 128:256], s64v[64:65, 512:1024],
                    start=True, stop=True,
                )
                bcR = expp.tile([128, 1024], f32, tag="bcR", bufs=2, name="bcR")
                nc.vector.reciprocal_approx_fast(bcR, bc)
                qs = slice(qc * 512, (qc + 1) * 512)
                for r, (stage, off) in enumerate(zip(stages.pop((hp, qc)), (0, 64))):
                    nc.vector.tensor_mul(
                        stage, stage, bcR[0:D, r * 512:(r + 1) * 512]
                    )
                    # sync queue: keeps the in-order gpsimd queue free for the
                    # next block's sums broadcast (convoy breaker)
                    nc.sync.dma_start(out=yT_t[hp][off:off + 64, qs], in_=stage)

            def FIN_mini(tau):
                """Sub-tile finish for the last beta block: query tile tau's
                attv columns are final once A(5,1,kt=tau) completes, so
                normalize and ship just those 128 columns, overlapped under
                the remaining attention.  Broadcast via two K=1 PE matmuls,
                reciprocal, fused normalize-copy, DMA to yT."""
                lo = (tau - 4) * 128
                hi = lo + 128
                ypA, ypB = yps[(5, 1)]
                with nc.allow_low_precision(reason="sums rounding is benign"):
                    nc.vector.tensor_copy(s64v[64:65, lo:hi], ypA[D:DA, lo:hi])
                    nc.vector.tensor_copy(
                        s64v[64:65, 512 + lo:512 + hi], ypB[D:DA, lo:hi]
                    )
                bc = psc.tile([128, 1024], f32, tag="ps", name=f"bcm{tau}")
                nc.tensor.matmul(
                    bc[:, 0:128], onr[64:65, 0:128], s64v[64:65, lo:hi],
                    start=True, stop=True,
                )
                nc.tensor.matmul(
                    bc[:, 128:256], onr[64:65, 128:256],
                    s64v[64:65, 512 + lo:512 + hi],
                    start=True, stop=True,
                )
                bcm = expp.tile([128, 256], f32, tag="bcm", bufs=2, name="bcm")
                nc.vector.reciprocal_approx_fast(bcm, bc[:, 0:256])
                for (yp, off, c0), eng in zip(
                    ((ypA, 0, 0), (ypB, 64, 128)), (nc.gpsimd, nc.sync)
                ):
                    stg = expp.tile([D, 128], fp16, tag="ymini", bufs=4, name="ym")
                    nc.vector.tensor_mul(stg, yp[:D, lo:hi], bcm[0:D, c0:c0 + 128])
                    eng.dma_start(
                        out=yT_t[5][off:off + 64, 512 + lo:512 + hi], in_=stg
                    )
                if tau == 7:
                    del yps[(5, 1)]

            # ---------------- schedule ----------------
            # Head: DMA issue costs ~0.65us per descriptor per engine queue, so
            # split between sync (x stream), scalar (wq + constants) and
            # gpsimd (wv/wp).
            nc.vector.memset(onr[64:65, :], 1.0)
            for m in (0, 6):
                wq_fetch(m, nc.scalar, split=True)
            for kk in range(NK):
                nc.sync.dma_start(out=xT_t[kk][:, 0:512], in_=xTd[:, kk, 0:512])
            for m in (1, 7):
                wq_fetch(m, nc.scalar)
            nc.scalar.dma_start(out=msk_t, in_=msk[:, :])
            nc.scalar.dma_start(out=ones_t, in_=onesc[:, :])
            for kk in range(3):
                nc.gpsimd.dma_start(out=xT_t[kk][:, 512:1024], in_=xTd[:, kk, 512:1024])
            for kk in range(3, NK):
                nc.sync.dma_start(out=xT_t[kk][:, 512:1024], in_=xTd[:, kk, 512:1024])
            nc.gpsimd.dma_start(out=wvd, in_=wvs)

            # ---- pre-alpha: the four QK tiles head pairs 0/1 need ----
            for m in (0, 6):
                drain(gen_qk(m))
            wq_fetch(2, nc.scalar)
            wq_fetch(8, nc.scalar)
            for m in (1, 7):
                drain(gen_qk(m))

            # ---- alpha: query half 0; remaining QK and v tiles as filler.
            # hp 0 runs a stretched weave that absorbs the v tiles ----
            alpha_jobs = {
                0: [gen_v(0), gen_v(1), gen_v(2), gen_v(3)],
                1: [gen_qk(2), gen_qk(8)],
                2: [gen_qk(3), gen_qk(9)],
                3: [gen_qk(4), gen_qk(10)],
                4: [gen_qk(5), gen_qk(11)],
                5: [gen_v(4), gen_v(5), gen_v(6)],
            }
            for hp in range(NK):
                # lazily bind jobs (gen_qk reads wq_tiles at first next())
                gens = alpha_jobs[hp]

                def fill():
                    # NB: the final next() of a generator runs its last chunk
                    # and THEN raises StopIteration, so pop-and-return.
                    if gens:
                        try:
                            next(gens[0])
                        except StopIteration:
                            gens.pop(0)

                xf = 1 if hp == 0 else 0  # extra fills while absorbing v tiles
                S(hp, 0, 0)
                S(hp, 0, 1)
                fill()
                fill()
                for _ in range(xf):
                    fill()
                A(hp, 0, 0, 4)
                S(hp, 0, 2)
                fill()
                for _ in range(2 * xf):
                    fill()
                A(hp, 0, 1, 4)
                S(hp, 0, 3)
                fill()
                fill()
                for _ in range(xf):
                    fill()
                A(hp, 0, 2, 4)
                if hp > 1:
                    FIN_normB(hp - 2, 0)
                fill()
                for _ in range(2 * xf):
                    fill()
                A(hp, 0, 3, 4)
                FIN_sums(hp, 0)
                while gens:
                    fill()
                if hp == 0:
                    wq_fetch(3)
                    wq_fetch(9)
                elif hp == 1:
                    wq_fetch(4)
                    wq_fetch(10)
                elif hp == 2:
                    wq_fetch(5)
                    wq_fetch(11)
                elif hp == 3:
                    nc.gpsimd.dma_start(
                        out=wpall.rearrange("p (i n) -> p i n", i=NK),
                        in_=wp.rearrange("i p n -> p i n"),
                    )

            # ---- beta: query half 1 attention + projection filler ----
            beta_jobs = {
                0: [gen_v(7), gen_P(0, 0, 3, "part"), gen_P(1, 0, 3, "part")],
                1: [gen_P(2, 0, 3, "part"), gen_P(3, 0, 3, "part"),
                    gen_P(0, 3, 6, "out")],
                2: [gen_P(1, 3, 6, "out"), gen_P(2, 3, 6, "out")],
                3: [gen_P(3, 3, 6, "out"), gen_P(4, 0, 3, "part"),
                    gen_P(5, 0, 3, "part")],
                4: [gen_P(6, 0, 3, "part"), gen_P(7, 0, 3, "part")],
                5: [gen_noop(2), gen_P(4, 3, 5, "acc"), gen_P(5, 3, 5, "acc"),
                    gen_P(6, 3, 5, "acc"), gen_P(7, 3, 5, "acc")],
            }
            for hp in range(NK):
                gens = beta_jobs[hp]

                def fill():
                    # NB: the final next() of a generator runs its last chunk
                    # and THEN raises StopIteration, so pop-and-return.
                    if gens:
                        try:
                            next(gens[0])
                        except StopIteration:
                            gens.pop(0)

                S(hp, 1, 0)
                S(hp, 1, 1)
                fill()
                fill()
                A(hp, 1, 0, 8)
                if hp == 5:
                    FIN_normB(4, 1)
                elif hp == 3:
                    FIN_normB(2, 1)
                S(hp, 1, 2)
                fill()
                A(hp, 1, 1, 8)
                S(hp, 1, 3)
                fill()
                A(hp, 1, 2, 8)
                if hp == 0:
                    FIN_normB(5, 0)
                elif hp < 5 and hp != 3:
                    FIN_normB(hp - 1, 1)
                S(hp, 1, 4)
                fill()
                A(hp, 1, 3, 8)
                S(hp, 1, 5)
                fill()
                A(hp, 1, 4, 8)
                if hp == 5:
                    FIN_mini(4)
                S(hp, 1, 6)
                fill()
                A(hp, 1, 5, 8)
                if hp == 5:
                    FIN_mini(5)
                    drain(gen_P(4, 5, 6, "out", nc.scalar))
                S(hp, 1, 7)
                fill()
                A(hp, 1, 6, 8)
                if hp == 5:
                    FIN_mini(6)
                    drain(gen_P(5, 5, 6, "out", nc.sync))
                fill()
                A(hp, 1, 7, 8)
                if hp == 5:
                    FIN_mini(7)
                    while gens:
                        fill()
                    drain(gen_P(6, 5, 6, "out", nc.scalar))
                    drain(gen_P(7, 5, 6, "out", nc.sync))
                else:
                    FIN_sums(hp, 1)
                    while gens:
                        fill()

    nc.compile()
    return nc


_nc = None


def _get_nc():
    global _nc
    if _nc is None:
        _nc = build()
    return _nc


def _host_prep(w_attn, w_proj):
    wq = np.ascontiguousarray(
        w_attn[:, :2 * C].reshape(NK, 128, 2 * NK, 128).transpose(2, 1, 0, 3)
    ).astype(np.float16)
    wv_aug = np.zeros((C, H, DA), np.float32)
    wv_aug[:, :, :D] = w_attn[:, 2 * C:].reshape(C, H, D)
    wv = np.ascontiguousarray(wv_aug.reshape(NK, 128, VW)).astype(np.float16)
    wp = np.ascontiguousarray(w_proj.reshape(NK, 128, C)).astype(np.float16)
    tri = np.triu(np.ones((128, 128), np.float32))
    msk = np.concatenate([tri, tri], axis=1).astype(np.float16)
    onesc = np.ones((128, H), np.float16)
    return wq, wv, wp, msk, onesc


def kernel(x, w_attn, w_proj):
    x = np.asarray(x, dtype=np.float32)
    w_attn = np.asarray(w_attn, dtype=np.float32)
    w_proj = np.asarray(w_proj, dtype=np.float32)
    wq, wv, wp, msk, onesc = _host_prep(w_attn, w_proj)
    in_maps = [
        {
            "xT": np.ascontiguousarray(x[b].T).astype(np.float16),
            "wq": wq,
            "wv": wv,
            "wp": wp,
            "msk": msk,
            "onesc": onesc,
        }
        for b in range(B)
    ]
    last_err = None
    for _attempt in range(3):
        try:
            res = run_bass_kernel_spmd(_get_nc(), in_maps, list(range(B)))
            return np.stack([res.results[b]["out"] for b in range(B)], axis=0)
        except Exception as e:  # transient device wedge: retry
            last_err = e
    raise last_err


# revision 36
# speedup vs baseline: 1.0111x; 1.0111x over previous
"""Causal self-attention Trainium2 kernel (B=8, T=1024, C=768, H=12 heads).

Strategy: data-parallel over batch — one batch element per NeuronCore (8 cores).
Per core, everything is computed in a "transposed" layout so that no on-device
transposes are needed:

  qT, kT  [C, T]   = w_attn_{q,k}.T @ x.T          (x.T supplied by host)
  v_aug   [T, 780] = x @ [w_attn_v | 0]  (+ ones column per head, stride 65)
  sT_h    [Tk, Tq] = kT_h.T-slices @ qT_h          (keys on partitions, the two
                                                    heads of a pair run as
                                                    concurrent row-tiled MMs)
  eT      = exp(sT / 8), fp16, causal mask via one batched 2-head multiply
  yT_aug  [65, Tq] = v_aug_h.T @ eT                (row 64 = softmax row-sums)
  yT_norm = yT * broadcast(1/sums)                 (broadcast via gpsimd
                                                    partition_broadcast)
  out     [T, C]   = yT_norm.T-slices @ w_proj

All matmul operands are fp16 (1 col/cycle PE rate, fast weight loads, half the
DMA traffic of fp32); PSUM accumulation stays fp32, final output is fp32.

The issue order forms a software pipeline tuned so no engine starves: the
scalar-engine exp latency ((N+352)/1.2 ns + ~0.3us semaphore hops) is hidden
by weaving ~0.5us filler matmul chunks (QK / v / projection, expressed as
Python generators) between every score and attv step.  Phase alpha covers
query half 0 (+ all QK and most v tiles); phase beta covers query half 1 with
the output projection as filler, split by contraction (heads 0-2 -> fp16 SBUF
partial, 3-4 as late filler, 5 in the tail) so the post-attention tail stays
short.  Per-head-pair normalization is deferred by one block so the in-order
PE queue never waits on the sums chain.  Head DMAs are split between the
sync and scalar queues (~0.65us serial issue cost per descriptor per queue);
yT staging DMAs and the sum broadcasts ride the otherwise-idle gpsimd queue.
"""
import sys

sys.path.insert(0, "/opt/trn_rl_repo")

import numpy as np

import concourse.bass as bass
import concourse.bacc as bacc
import concourse.tile as tile
import concourse.mybir as mybir
from concourse.bass_utils import run_bass_kernel_spmd

f32 = mybir.dt.float32
fp16 = mybir.dt.float16
EXP = mybir.ActivationFunctionType.Exp

B, T, C = 8, 1024, 768
H, D = 12, 64
DA = D + 1        # per-head block in v: [v_h(64) | 1]
HB = 2 * DA       # head-pair stride
VW = H * DA       # 780
NK = C // 128     # 6 contraction tiles
NT = T // 128     # 8 token tiles
SCALE = 1.0 / np.sqrt(D)


def build():
    nc = bacc.Bacc("TRN2", target_bir_lowering=False, debug=False)
    xT = nc.dram_tensor("xT", [C, T], fp16, kind="ExternalInput")
    wq = nc.dram_tensor("wq", [2 * NK, 128, NK, 128], fp16, kind="ExternalInput")
    wv = nc.dram_tensor("wv", [NK, 128, VW], fp16, kind="ExternalInput")
    wp = nc.dram_tensor("wp", [NK, 128, C], fp16, kind="ExternalInput")
    msk = nc.dram_tensor("msk", [128, 256], fp16, kind="ExternalInput")
    onesc = nc.dram_tensor("onesc", [128, H], fp16, kind="ExternalInput")
    out = nc.dram_tensor("out", [T, C], f32, kind="ExternalOutput")

    with tile.TileContext(nc) as tc:
        with (
            tc.tile_pool(name="const", bufs=1) as const,
            tc.tile_pool(name="wqp", bufs=4) as wqp,
            tc.tile_pool(name="exp", bufs=4) as expp,
            tc.tile_pool(name="psc", bufs=3, space="PSUM") as psc,
            tc.tile_pool(name="psm", bufs=2, space="PSUM") as psm,
        ):
            # ---- resident SBUF tensors ----
            xTall = const.tile([128, NK * T], fp16, tag="xTall")
            xT_t = [xTall[:, i * T:(i + 1) * T] for i in range(NK)]
            wvall = const.tile([128, NK * VW], fp16, tag="wvall")
            wv_t = [wvall[:, i * VW:(i + 1) * VW] for i in range(NK)]
            wvd = wvall.rearrange("p (i n) -> p i n", i=NK)
            wpall = const.tile([128, NK * C], fp16, tag="wpall")
            wp_t = [wpall[:, i * C:(i + 1) * C] for i in range(NK)]
            qkT_t = [const.tile([128, T], fp16, name=f"qks{m}", tag=f"qk{m}") for m in range(2 * NK)]
            v_t = [const.tile([128, VW], fp16, name=f"vs{t}", tag=f"v{t}") for t in range(NT)]
            yT_t = [const.tile([128, T], fp16, name=f"yTs{i}", tag=f"yT{i}") for i in range(NK)]
            part = [const.tile([128, C], fp16, name=f"prt{t}", tag=f"prt{t}") for t in range(NT)]
            msk_t = const.tile([128, 256], fp16, tag="msk")
            ones_t = const.tile([128, H], fp16, tag="ones")
            # softmax sums sit on PSUM row 64 of the attv output; a tiny
            # gpsimd DMA drops them onto partition 0 of s64r (head A at cols
            # 0:512, head B 512:1024).  They are then broadcast to all 128
            # partitions by two gpsimd partition_broadcast ops (partition-0
            # in / partition-0 out — the only HW-supported form), and the DVE
            # reciprocal + multiply normalize the staged y tiles in place
            # before the DMA into yT.
            s64v = const.tile([65, 1024], f32, tag="s64v")
            s64r = const.tile([1, 1024], f32, tag="s64r")
            onr = const.tile([65, 256], f32, tag="onr")

            xTd = xT.rearrange("(i p) n -> p i n", p=128)
            wvs = wv.rearrange("i p n -> p i n")

            wq_tiles = {}

            def wq_fetch(m, eng=None, split=False):
                wq_tiles[m] = wqp.tile([128, NK, 128], fp16, tag="wq", name=f"wq{m}")
                if split:  # halves: first matmuls start before the tail lands
                    (eng or nc.sync).dma_start(
                        out=wq_tiles[m][:, 0:3, :], in_=wq[m, :, 0:3, :]
                    )
                    (eng or nc.sync).dma_start(
                        out=wq_tiles[m][:, 3:NK, :], in_=wq[m, :, 3:NK, :]
                    )
                else:
                    (eng or nc.sync).dma_start(out=wq_tiles[m], in_=wq[m, :, :, :])

            # ---------------- building blocks (filler jobs are generators;
            # each `yield` is a ~0.5us chunk boundary for the weave) ---------
            def gen_qk(m):
                wq_t = wq_tiles[m]
                ps = psc.tile([128, 1024], f32, tag="ps", name=f"psqk{m}")

                def mm(qc, kk):
                    nc.tensor.matmul(
                        ps[:, qc * 512:(qc + 1) * 512],
                        wq_t[:, kk, :],
                        xT_t[kk][:, qc * 512:(qc + 1) * 512],
                        start=(kk == 0),
                        stop=(kk == NK - 1),
                    )

                def cp(half):
                    dst = qkT_t[m][:, half * 512:(half + 1) * 512]
                    src = ps[:, half * 512:(half + 1) * 512]
                    if m % 2 == 0:
                        nc.scalar.copy(dst, src)
                    else:
                        nc.vector.tensor_copy(dst, src)

                for kk in range(4):
                    mm(0, kk)
                yield
                for kk in range(4, NK):
                    mm(0, kk)
                cp(0)  # first half drains while the second accumulates
                for kk in range(2):
                    mm(1, kk)
                yield
                for kk in range(2, NK):
                    mm(1, kk)
                wq_tiles.pop(m)
                cp(1)

            def gen_v(t):
                ps = psc.tile([128, 1024], f32, tag="ps", name=f"psv{t}")
                for ci, kks in enumerate(((0, 1), (2, 3), (4, 5))):
                    for kk in kks:
                        for n0, nw in ((0, 512), (512, VW - 512)):
                            nc.tensor.matmul(
                                ps[:, n0:n0 + nw],
                                xT_t[kk][:, t * 128:(t + 1) * 128],
                                wv_t[kk][:, n0:n0 + nw],
                                start=(kk == 0),
                                stop=(kk == NK - 1),
                            )
                    if ci < 2:
                        yield
                if t % 2 == 0:
                    nc.scalar.copy(v_t[t], ps[:, :VW])
                else:
                    nc.vector.tensor_copy(v_t[t], ps[:, :VW])
                ones_ap = v_t[t].rearrange("p (h e) -> p h e", e=DA)[:, :, D]
                nc.vector.tensor_copy(ones_ap, ones_t)

            def gen_P(t, kk0, kk1, mode, eng=None):
                """Projection tile t over contraction tiles [kk0, kk1).
                mode: 'part' -> write fp16 partial; 'acc' -> add into partial;
                'out' -> add partial + DMA the finished row block out (on
                engine `eng`, default sync)."""
                pp = psc.tile([128, 1024], f32, tag="ps", name=f"pp{t}_{kk0}")
                for kk in range(kk0, kk1):
                    nc.tensor.matmul(
                        pp[:, 0:512],
                        yT_t[kk][:, t * 128:(t + 1) * 128],
                        wp_t[kk][:, 0:512],
                        start=(kk == kk0),
                        stop=(kk == kk1 - 1),
                    )
                yield
                for kk in range(kk0, kk1):
                    nc.tensor.matmul(
                        pp[:, 512:768],
                        yT_t[kk][:, t * 128:(t + 1) * 128],
                        wp_t[kk][:, 512:768],
                        start=(kk == kk0),
                        stop=(kk == kk1 - 1),
                    )
                if mode == "part":
                    nc.vector.tensor_copy(part[t], pp[:, :C])
                elif mode == "acc":
                    nc.vector.tensor_add(part[t], pp[:, :C], part[t])
                else:
                    ostage = expp.tile([128, C], f32, tag="ostage", bufs=2, name="ostage")
                    for h0, h1 in ((0, 384), (384, C)):
                        nc.vector.tensor_add(
                            ostage[:, h0:h1], pp[:, h0:h1], part[t][:, h0:h1]
                        )
                        (eng or nc.sync).dma_start(
                            out=out[t * 128:(t + 1) * 128, h0:h1], in_=ostage[:, h0:h1]
                        )

            def gen_noop(n):
                for _ in range(n - 1):
                    yield

            def drain(g):
                for _ in g:
                    pass

            yps = {}
            exs = {}

            def S(hp, qc, kt):
                """Scores pair (row-tiled, concurrent) + exp (+ causal mask)."""
                qT = qkT_t[hp]
                kT = qkT_t[NK + hp]
                ks = slice(kt * 128, (kt + 1) * 128)
                pos = max(kt * 128 - qc * 512, 0)
                qv = slice(qc * 512 + pos, (qc + 1) * 512)
                sp = psc.tile([128, 1024], f32, tag="ps", name="sp")
                nc.tensor.matmul(
                    sp[:, pos:512], kT[0:64, ks], qT[0:64, qv],
                    start=True, stop=True,
                )
                nc.tensor.matmul(
                    sp[:, 512 + pos:1024], kT[64:128, ks], qT[64:128, qv],
                    start=True, stop=True,
                )
                ex = expp.tile([128, 1024], fp16, tag="ex", bufs=8, name="ex")
                if pos == 0:
                    nc.scalar.activation(ex, sp, EXP, scale=float(SCALE))
                else:
                    exv = ex.rearrange("p (i n) -> p i n", i=2)[:, :, pos:512]
                    spv = sp.rearrange("p (i n) -> p i n", i=2)[:, :, pos:512]
                    nc.scalar.activation(exv, spv, EXP, scale=float(SCALE))
                if kt * 128 >= qc * 512:  # diagonal tile: mask both heads at once
                    exd = ex.rearrange("p (i n) -> p i n", i=2)[:, :, pos:pos + 128]
                    mkd = msk_t.rearrange("p (i n) -> p i n", i=2)
                    nc.vector.tensor_mul(exd, exd, mkd)
                exs[(hp, qc, kt)] = (ex, pos)

            def A(hp, qc, kt, nkt):
                """attv pair for exp tile (hp, qc, kt)."""
                if (hp, qc) not in yps:
                    yps[(hp, qc)] = (
                        psm.tile([128, 512], f32, tag="yp", name="ypA"),
                        psm.tile([128, 512], f32, tag="yp", name="ypB"),
                    )
                ypA, ypB = yps[(hp, qc)]
                ex, pos = exs.pop((hp, qc, kt))
                for yp, half in ((ypA, 0), (ypB, 1)):
                    nc.tensor.matmul(
                        yp[:DA, pos:512],
                        v_t[kt][:, hp * HB + half * DA:hp * HB + (half + 1) * DA],
                        ex[:, half * 512 + pos:(half + 1) * 512],
                        start=(kt == 0), stop=(kt == nkt - 1),
                    )

            stages = {}
            bcast = {}

            def FIN_sums(hp, qc, tail=False):
                """Issued right after the last attv of the block: pull the
                softmax sums out of PSUM row 64 (DVE copy — DMA cannot read
                PSUM), drop them onto partition 0 of s64r with one gpsimd
                SBUF-to-SBUF DMA, and broadcast them to all partitions.  This
                jumps the vector queue ahead of the block-tail casts so the
                broadcast is long done when FIN_normB consumes it."""
                with nc.allow_low_precision(reason="sums rounding is benign"):
                    for r, yp in enumerate(yps[(hp, qc)]):
                        nc.vector.tensor_copy(
                            s64v[64:65, r * 512:(r + 1) * 512], yp[D:DA, :]
                        )
                if tail:
                    return
                nc.gpsimd.dma_start(out=s64r[0:1, :], in_=s64v[64:65, :])
                bcS = expp.tile([128, 1024], f32, tag="bcS", bufs=2, name="bcS")
                nc.gpsimd.partition_broadcast(bcS, s64r[0:1, :], channels=128)
                bcast[(hp, qc)] = bcS
                # stage the unnormalized y rows right away: the copies drain
                # on the vector queue during the block-tail fills, freeing the
                # psum pair before the next block's first attv (WAR) and
                # keeping the next block's mask muls unqueued.
                st = []
                for yp in yps[(hp, qc)]:
                    stage = expp.tile([D, 512], fp16, tag="ystage", bufs=4, name="stage")
                    nc.vector.tensor_copy(stage, yp[:D, :])
                    st.append(stage)
                stages[(hp, qc)] = st
                del yps[(hp, qc)]

            def FIN_normB(hp, qc):
                """Reciprocal of the broadcast sums, normalize the staged y
                tiles in place, DMA them into yT.  Deferred to the middle of
                the next block so the in-order vector queue never waits on
                the gpsimd broadcast."""
                bcS = bcast.pop((hp, qc))
                bcR = expp.tile([128, 1024], f32, tag="bcR", bufs=2, name="bcR")
                nc.vector.reciprocal_approx_fast(bcR, bcS)
                qs = slice(qc * 512, (qc + 1) * 512)
                for r, (stage, off) in enumerate(zip(stages.pop((hp, qc)), (0, 64))):
                    nc.vector.tensor_mul(
                        stage, stage, bcR[0:D, r * 512:(r + 1) * 512]
                    )
                    # sync queue: keeps the in-order gpsimd queue free for the
                    # next block's sums broadcast (convoy breaker)
                    nc.sync.dma_start(out=yT_t[hp][off:off + 64, qs], in_=stage)

            def FIN_tail(hp, qc):
                """Fast finish for the very last block: broadcast the sums
                with two K=1 PE matmuls (the PE is idle in the tail, and this
                skips the long gpsimd broadcast chain), then reciprocal,
                normalize, ship."""
                bc = psc.tile([128, 1024], f32, tag="ps", name="bc")
                nc.tensor.matmul(
                    bc[:, 0:512], onr[64:65, 0:128], s64v[64:65, 0:512],
                    start=True, stop=True,
                )
                nc.tensor.matmul(
                    bc[:, 512:1024], onr[64:65, 128:256], s64v[64:65, 512:1024],
                    start=True, stop=True,
                )
                bcR = expp.tile([128, 1024], f32, tag="bcR", bufs=2, name="bcR")
                nc.vector.reciprocal_approx_fast(bcR, bc)
                qs = slice(qc * 512, (qc + 1) * 512)
                for r, (stage, off) in enumerate(zip(stages.pop((hp, qc)), (0, 64))):
                    nc.vector.tensor_mul(
                        stage, stage, bcR[0:D, r * 512:(r + 1) * 512]
                    )
                    # sync queue: keeps the in-order gpsimd queue free for the
                    # next block's sums broadcast (convoy breaker)
                    nc.sync.dma_start(out=yT_t[hp][off:off + 64, qs], in_=stage)

            def FIN_mini(tau):
                """Sub-tile finish for the last beta block: query tile tau's
                attv columns are final once A(5,1,kt=tau) completes, so
                normalize and ship just those 128 columns, overlapped under
                the remaining attention.  Broadcast via two K=1 PE matmuls,
                reciprocal, fused normalize-copy, DMA to yT."""
                lo = (tau - 4) * 128
                hi = lo + 128
                ypA, ypB = yps[(5, 1)]
                with nc.allow_low_precision(reason="sums rounding is benign"):
                    nc.vector.tensor_copy(s64v[64:65, lo:hi], ypA[D:DA, lo:hi])
                    nc.vector.tensor_copy(
                        s64v[64:65, 512 + lo:512 + hi], ypB[D:DA, lo:hi]
                    )
                bc = psc.tile([128, 1024], f32, tag="ps", name=f"bcm{tau}")
                nc.tensor.matmul(
                    bc[:, 0:128], onr[64:65, 0:128], s64v[64:65, lo:hi],
                    start=True, stop=True,
                )
                nc.tensor.matmul(
                    bc[:, 128:256], onr[64:65, 128:256],
                    s64v[64:65, 512 + lo:512 + hi],
                    start=True, stop=True,
                )
                bcm = expp.tile([128, 256], f32, tag="bcm", bufs=2, name="bcm")
                nc.vector.reciprocal_approx_fast(bcm, bc[:, 0:256])
                for (yp, off, c0), eng in zip(
                    ((ypA, 0, 0), (ypB, 64, 128)), (nc.gpsimd, nc.sync)
                ):
                    stg = expp.tile([D, 128], fp16, tag="ymini", bufs=4, name="ym")
                    nc.vector.tensor_mul(stg, yp[:D, lo:hi], bcm[0:D, c0:c0 + 128])
                    eng.dma_start(
                        out=yT_t[5][off:off + 64, 512 + lo:512 + hi], in_=stg
                    )
                if tau == 7:
                    del yps[(5, 1)]

            # ---------------- schedule ----------------
            # Head: DMA issue costs ~0.65us per descriptor per engine queue, so
            # split between sync (x stream), scalar (wq + constants) and
            # gpsimd (wv/wp).
            nc.vector.memset(onr[64:65, :], 1.0)
            for m in (0, 6):
                wq_fetch(m, nc.scalar, split=True)
            for kk in range(NK):
                nc.sync.dma_start(out=xT_t[kk][:, 0:512], in_=xTd[:, kk, 0:512])
            for m in (1, 7):
                wq_fetch(m, nc.scalar)
            nc.scalar.dma_start(out=msk_t, in_=msk[:, :])
            nc.scalar.dma_start(out=ones_t, in_=onesc[:, :])
            for kk in range(3):
                nc.gpsimd.dma_start(out=xT_t[kk][:, 512:1024], in_=xTd[:, kk, 512:1024])
            for kk in range(3, NK):
                nc.sync.dma_start(out=xT_t[kk][:, 512:1024], in_=xTd[:, kk, 512:1024])
            nc.gpsimd.dma_start(out=wvd, in_=wvs)

            # ---- pre-alpha: the four QK tiles head pairs 0/1 need ----
            for m in (0, 6):
                drain(gen_qk(m))
            wq_fetch(2, nc.scalar)
            wq_fetch(8, nc.scalar)
            for m in (1, 7):
                drain(gen_qk(m))

            # ---- alpha: query half 0; remaining QK and v tiles as filler.
            # hp 0 runs a stretched weave that absorbs the v tiles ----
            alpha_jobs = {
                0: [gen_v(0), gen_v(1), gen_v(2), gen_v(3)],
                1: [gen_qk(2), gen_qk(8)],
                2: [gen_qk(3), gen_qk(9)],
                3: [gen_qk(4), gen_qk(10)],
                4: [gen_qk(5), gen_qk(11)],
                5: [gen_v(4), gen_v(5), gen_v(6)],
            }
            for hp in range(NK):
                # lazily bind jobs (gen_qk reads wq_tiles at first next())
                gens = alpha_jobs[hp]

                def fill():
                    # NB: the final next() of a generator runs its last chunk
                    # and THEN raises StopIteration, so pop-and-return.
                    if gens:
                        try:
                            next(gens[0])
                        except StopIteration:
                            gens.pop(0)

                xf = 1 if hp == 0 else 0  # extra fills while absorbing v tiles
                S(hp, 0, 0)
                S(hp, 0, 1)
                fill()
                fill()
                for _ in range(xf):
                    fill()
                A(hp, 0, 0, 4)
                S(hp, 0, 2)
                fill()
                for _ in range(2 * xf):
                    fill()
                A(hp, 0, 1, 4)
                S(hp, 0, 3)
                fill()
                fill()
                for _ in range(xf):
                    fill()
                A(hp, 0, 2, 4)
                if hp > 1:
                    FIN_normB(hp - 2, 0)
                fill()
                for _ in range(2 * xf):
                    fill()
                A(hp, 0, 3, 4)
                FIN_sums(hp, 0)
                while gens:
                    fill()
                if hp == 0:
                    wq_fetch(3)
                    wq_fetch(9)
                elif hp == 1:
                    wq_fetch(4)
                    wq_fetch(10)
                elif hp == 2:
                    wq_fetch(5)
                    wq_fetch(11)
                elif hp == 3:
                    nc.gpsimd.dma_start(
                        out=wpall.rearrange("p (i n) -> p i n", i=NK),
                        in_=wp.rearrange("i p n -> p i n"),
                    )

            # ---- beta: query half 1 attention + projection filler ----
            beta_jobs = {
                0: [gen_v(7), gen_P(0, 0, 3, "part"), gen_P(1, 0, 3, "part")],
                1: [gen_P(2, 0, 3, "part"), gen_P(3, 0, 3, "part"),
                    gen_P(0, 3, 6, "out")],
                2: [gen_P(1, 3, 6, "out"), gen_P(2, 3, 6, "out")],
                3: [gen_P(3, 3, 6, "out"), gen_P(4, 0, 3, "part"),
                    gen_P(5, 0, 3, "part")],
                4: [gen_P(6, 0, 3, "part"), gen_P(7, 0, 3, "part")],
                5: [gen_noop(2), gen_P(4, 3, 5, "acc"), gen_P(5, 3, 5, "acc"),
                    gen_P(6, 3, 5, "acc"), gen_P(7, 3, 5, "acc")],
            }
            for hp in range(NK):
                gens = beta_jobs[hp]

                def fill():
                    # NB: the final next() of a generator runs its last chunk
                    # and THEN raises StopIteration, so pop-and-return.
                    if gens:
                        try:
                            next(gens[0])
                        except StopIteration:
                            gens.pop(0)

                S(hp, 1, 0)
                S(hp, 1, 1)
                fill()
                fill()
                A(hp, 1, 0, 8)
                if hp == 5:
                    FIN_normB(4, 1)
                elif hp == 3:
                    FIN_normB(2, 1)
                S(hp, 1, 2)
                fill()
                A(hp, 1, 1, 8)
                S(hp, 1, 3)
                fill()
                A(hp, 1, 2, 8)
                if hp == 0:
                    FIN_normB(5, 0)
                elif hp < 5 and hp != 3:
                    FIN_normB(hp - 1, 1)
                S(hp, 1, 4)
                fill()
                A(hp, 1, 3, 8)
                S(hp, 1, 5)
                fill()
                A(hp, 1, 4, 8)
                if hp == 5:
                    FIN_mini(4)
                S(hp, 1, 6)
                fill()
                A(hp, 1, 5, 8)
                if hp == 5:
                    FIN_mini(5)
                    drain(gen_P(4, 5, 6, "out", nc.scalar))
                S(hp, 1, 7)
                fill()
                A(hp, 1, 6, 8)
                if hp == 5:
                    FIN_mini(6)
                    drain(gen_P(5, 5, 6, "out", nc.sync))
                fill()
                A(hp, 1, 7, 8)
                if hp == 5:
                    FIN_mini(7)
                    while gens:
                        fill()
                    drain(gen_P(6, 5, 6, "out", nc.scalar))
                    drain(gen_P(7, 5, 6, "out", nc.sync))
                else:
                    FIN_sums(hp, 1)
                    while gens:
                        fill()

    nc.compile()
    return nc


_nc = None


def _get_nc():
    global _nc
    if _nc is None:
        _nc = build()
    return _nc


def _host_prep(w_attn, w_proj):
    wq = np.ascontiguousarray(
        w_attn[:, :2 * C].reshape(NK, 128, 2 * NK, 128).transpose(2, 1, 0, 3)
    ).astype(np.float16)
    wv_aug = np.zeros((C, H, DA), np.float32)
    wv_aug[:, :, :D] = w_attn[:, 2 * C:].reshape(C, H, D)
    wv = np.ascontiguousarray(wv_aug.reshape(NK, 128, VW)).astype(np.float16)
    wp = np.ascontiguousarray(w_proj.reshape(NK, 128, C)).astype(np.float16)
    tri = np.triu(np.ones((128, 128), np.float32))
    msk = np.concatenate([tri, tri], axis=1).astype(np.float16)
    onesc = np.ones((128, H), np.float16)
    return wq, wv, wp, msk, onesc


def kernel(x, w_attn, w_proj):
    x = np.asarray(x, dtype=np.float32)
    w_attn = np.asarray(w_attn, dtype=np.float32)
    w_proj = np.asarray(w_proj, dtype=np.float32)
    wq, wv, wp, msk, onesc = _host_prep(w_attn, w_proj)
    in_maps = [
        {
            "xT": np.ascontiguousarray(x[b].T).astype(np.float16),
            "wq": wq,
            "wv": wv,
            "wp": wp,
            "msk": msk,
            "onesc": onesc,
        }
        for b in range(B)
    ]
    last_err = None
    for _attempt in range(3):
        try:
            res = run_bass_kernel_spmd(_get_nc(), in_maps, list(range(B)))
            return np.stack([res.results[b]["out"] for b in range(B)], axis=0)
        except Exception as e:  # transient device wedge: retry
            last_err = e
    raise last_err


# revision 37
# speedup vs baseline: 1.0241x; 1.0128x over previous
"""Causal self-attention Trainium2 kernel (B=8, T=1024, C=768, H=12 heads).

Strategy: data-parallel over batch — one batch element per NeuronCore (8 cores).
Per core, everything is computed in a "transposed" layout so that no on-device
transposes are needed:

  qT, kT  [C, T]   = w_attn_{q,k}.T @ x.T          (x.T supplied by host)
  v_aug   [T, 780] = x @ [w_attn_v | 0]  (+ ones column per head, stride 65)
  sT_h    [Tk, Tq] = kT_h.T-slices @ qT_h          (keys on partitions, the two
                                                    heads of a pair run as
                                                    concurrent row-tiled MMs)
  eT      = exp(sT / 8), fp16, causal mask via one batched 2-head multiply
  yT_aug  [65, Tq] = v_aug_h.T @ eT                (row 64 = softmax row-sums)
  yT_norm = yT * broadcast(1/sums)                 (broadcast via gpsimd
                                                    partition_broadcast)
  out     [T, C]   = yT_norm.T-slices @ w_proj

All matmul operands are fp16 (1 col/cycle PE rate, fast weight loads, half the
DMA traffic of fp32); PSUM accumulation stays fp32, final output is fp32.

The issue order forms a software pipeline tuned so no engine starves: the
scalar-engine exp latency ((N+352)/1.2 ns + ~0.3us semaphore hops) is hidden
by weaving filler matmul chunks (QK / v / projection, expressed as Python
generators) between every score and attv step.  Phase alpha covers query
half 0 (absorbing the v tiles into the stretched head-pair-0 block); phase
beta covers query half 1 with the output projection as filler.

Normalization pipeline per head-pair block: FIN_sums (right after the last
attv) copies the sums row out of PSUM, hops it to partition 0 with a tiny
gpsimd SBUF-to-SBUF DMA, broadcasts it (gpsimd partition_broadcast — which
on HW only supports partition-0 in / partition-0 out), and stages the
unnormalized y rows; FIN_normB (one block later in alpha+2 for the early
pairs) does reciprocal + in-place normalize + DMA into yT.  The stage->yT
DMAs ride the mid-run-idle sync queue: putting them on gpsimd creates an
in-order convoy that delays every later broadcast behind vector-gated DMAs.
The last beta block finishes per 128-query sub-tile (FIN_mini): a column
range of the attv PSUM is final once its diagonal key tile is accumulated
(skip_group_check — the accumulation-group bookkeeping is sim-only), so the
reciprocal/normalize/projection/output-DMA chain for early sub-tiles hides
under the remaining attention, leaving only the last sub-tile as true tail.
Head DMAs are split across the sync/scalar/gpsimd queues (~0.65us serial
issue cost per descriptor per queue), with the first weight tiles split in
half so the first matmuls start before the tails land.
"""
import sys

sys.path.insert(0, "/opt/trn_rl_repo")

import numpy as np

import concourse.bass as bass
import concourse.bacc as bacc
import concourse.tile as tile
import concourse.mybir as mybir
from concourse.bass_utils import run_bass_kernel_spmd

f32 = mybir.dt.float32
fp16 = mybir.dt.float16
EXP = mybir.ActivationFunctionType.Exp

B, T, C = 8, 1024, 768
H, D = 12, 64
DA = D + 1        # per-head block in v: [v_h(64) | 1]
HB = 2 * DA       # head-pair stride
VW = H * DA       # 780
NK = C // 128     # 6 contraction tiles
NT = T // 128     # 8 token tiles
SCALE = 1.0 / np.sqrt(D)


def build():
    nc = bacc.Bacc("TRN2", target_bir_lowering=False, debug=False)
    xT = nc.dram_tensor("xT", [C, T], fp16, kind="ExternalInput")
    wq = nc.dram_tensor("wq", [2 * NK, 128, NK, 128], fp16, kind="ExternalInput")
    wv = nc.dram_tensor("wv", [NK, 128, VW], fp16, kind="ExternalInput")
    wp = nc.dram_tensor("wp", [NK, 128, C], fp16, kind="ExternalInput")
    msk = nc.dram_tensor("msk", [128, 256], fp16, kind="ExternalInput")
    onesc = nc.dram_tensor("onesc", [128, H], fp16, kind="ExternalInput")
    out = nc.dram_tensor("out", [T, C], f32, kind="ExternalOutput")

    with tile.TileContext(nc) as tc:
        with (
            tc.tile_pool(name="const", bufs=1) as const,
            tc.tile_pool(name="wqp", bufs=4) as wqp,
            tc.tile_pool(name="exp", bufs=4) as expp,
            tc.tile_pool(name="psc", bufs=3, space="PSUM") as psc,
            tc.tile_pool(name="psm", bufs=2, space="PSUM") as psm,
        ):
            # ---- resident SBUF tensors ----
            xTall = const.tile([128, NK * T], fp16, tag="xTall")
            xT_t = [xTall[:, i * T:(i + 1) * T] for i in range(NK)]
            wvall = const.tile([128, NK * VW], fp16, tag="wvall")
            wv_t = [wvall[:, i * VW:(i + 1) * VW] for i in range(NK)]
            wvd = wvall.rearrange("p (i n) -> p i n", i=NK)
            wpall = const.tile([128, NK * C], fp16, tag="wpall")
            wp_t = [wpall[:, i * C:(i + 1) * C] for i in range(NK)]
            qkT_t = [const.tile([128, T], fp16, name=f"qks{m}", tag=f"qk{m}") for m in range(2 * NK)]
            v_t = [const.tile([128, VW], fp16, name=f"vs{t}", tag=f"v{t}") for t in range(NT)]
            yT_t = [const.tile([128, T], fp16, name=f"yTs{i}", tag=f"yT{i}") for i in range(NK)]
            part = [const.tile([128, C], fp16, name=f"prt{t}", tag=f"prt{t}") for t in range(NT)]
            msk_t = const.tile([128, 256], fp16, tag="msk")
            ones_t = const.tile([128, H], fp16, tag="ones")
            # softmax sums sit on PSUM row 64 of the attv output; a tiny
            # gpsimd DMA drops them onto partition 0 of s64r (head A at cols
            # 0:512, head B 512:1024).  They are then broadcast to all 128
            # partitions by two gpsimd partition_broadcast ops (partition-0
            # in / partition-0 out — the only HW-supported form), and the DVE
            # reciprocal + multiply normalize the staged y tiles in place
            # before the DMA into yT.
            s64v = const.tile([65, 1024], f32, tag="s64v")
            s64r = const.tile([1, 1024], f32, tag="s64r")
            onr = const.tile([65, 256], f32, tag="onr")

            xTd = xT.rearrange("(i p) n -> p i n", p=128)
            wvs = wv.rearrange("i p n -> p i n")

            wq_tiles = {}

            def wq_fetch(m, eng=None, split=False):
                wq_tiles[m] = wqp.tile([128, NK, 128], fp16, tag="wq", name=f"wq{m}")
                if split:  # halves: first matmuls start before the tail lands
                    (eng or nc.sync).dma_start(
                        out=wq_tiles[m][:, 0:3, :], in_=wq[m, :, 0:3, :]
                    )
                    (eng or nc.sync).dma_start(
                        out=wq_tiles[m][:, 3:NK, :], in_=wq[m, :, 3:NK, :]
                    )
                else:
                    (eng or nc.sync).dma_start(out=wq_tiles[m], in_=wq[m, :, :, :])

            # ---------------- building blocks (filler jobs are generators;
            # each `yield` is a ~0.5us chunk boundary for the weave) ---------
            def gen_qk(m):
                wq_t = wq_tiles[m]
                ps = psc.tile([128, 1024], f32, tag="ps", name=f"psqk{m}")

                def mm(qc, kk):
                    nc.tensor.matmul(
                        ps[:, qc * 512:(qc + 1) * 512],
                        wq_t[:, kk, :],
                        xT_t[kk][:, qc * 512:(qc + 1) * 512],
                        start=(kk == 0),
                        stop=(kk == NK - 1),
                    )

                def cp(half):
                    dst = qkT_t[m][:, half * 512:(half + 1) * 512]
                    src = ps[:, half * 512:(half + 1) * 512]
                    if m % 2 == 0:
                        nc.scalar.copy(dst, src)
                    else:
                        nc.vector.tensor_copy(dst, src)

                for kk in range(4):
                    mm(0, kk)
                yield
                for kk in range(4, NK):
                    mm(0, kk)
                cp(0)  # first half drains while the second accumulates
                for kk in range(2):
                    mm(1, kk)
                yield
                for kk in range(2, NK):
                    mm(1, kk)
                wq_tiles.pop(m)
                cp(1)

            def gen_v(t):
                ps = psc.tile([128, 1024], f32, tag="ps", name=f"psv{t}")
                for ci, kks in enumerate(((0, 1), (2, 3), (4, 5))):
                    for kk in kks:
                        for n0, nw in ((0, 512), (512, VW - 512)):
                            nc.tensor.matmul(
                                ps[:, n0:n0 + nw],
                                xT_t[kk][:, t * 128:(t + 1) * 128],
                                wv_t[kk][:, n0:n0 + nw],
                                start=(kk == 0),
                                stop=(kk == NK - 1),
                            )
                    if ci < 2:
                        yield
                if t % 2 == 0:
                    nc.scalar.copy(v_t[t], ps[:, :VW])
                else:
                    nc.vector.tensor_copy(v_t[t], ps[:, :VW])
                ones_ap = v_t[t].rearrange("p (h e) -> p h e", e=DA)[:, :, D]
                nc.vector.tensor_copy(ones_ap, ones_t)

            def gen_P(t, kk0, kk1, mode, eng=None):
                """Projection tile t over contraction tiles [kk0, kk1).
                mode: 'part' -> write fp16 partial; 'acc' -> add into partial;
                'out' -> add partial + DMA the finished row block out (on
                engine `eng`, default sync)."""
                pp = psc.tile([128, 1024], f32, tag="ps", name=f"pp{t}_{kk0}")
                for kk in range(kk0, kk1):
                    nc.tensor.matmul(
                        pp[:, 0:512],
                        yT_t[kk][:, t * 128:(t + 1) * 128],
                        wp_t[kk][:, 0:512],
                        start=(kk == kk0),
                        stop=(kk == kk1 - 1),
                    )
                yield
                for kk in range(kk0, kk1):
                    nc.tensor.matmul(
                        pp[:, 512:768],
                        yT_t[kk][:, t * 128:(t + 1) * 128],
                        wp_t[kk][:, 512:768],
                        start=(kk == kk0),
                        stop=(kk == kk1 - 1),
                    )
                if mode == "part":
                    nc.vector.tensor_copy(part[t], pp[:, :C])
                elif mode == "acc":
                    nc.vector.tensor_add(part[t], pp[:, :C], part[t])
                else:
                    ostage = expp.tile([128, C], f32, tag="ostage", bufs=2, name="ostage")
                    for h0, h1 in ((0, 384), (384, C)):
                        nc.vector.tensor_add(
                            ostage[:, h0:h1], pp[:, h0:h1], part[t][:, h0:h1]
                        )
                        (eng or nc.sync).dma_start(
                            out=out[t * 128:(t + 1) * 128, h0:h1], in_=ostage[:, h0:h1]
                        )

            def gen_noop(n):
                for _ in range(n - 1):
                    yield

            def drain(g):
                for _ in g:
                    pass

            yps = {}
            exs = {}

            def S(hp, qc, kt):
                """Scores pair (row-tiled, concurrent) + exp (+ causal mask)."""
                qT = qkT_t[hp]
                kT = qkT_t[NK + hp]
                ks = slice(kt * 128, (kt + 1) * 128)
                pos = max(kt * 128 - qc * 512, 0)
                qv = slice(qc * 512 + pos, (qc + 1) * 512)
                sp = psc.tile([128, 1024], f32, tag="ps", name="sp")
                nc.tensor.matmul(
                    sp[:, pos:512], kT[0:64, ks], qT[0:64, qv],
                    start=True, stop=True,
                )
                nc.tensor.matmul(
                    sp[:, 512 + pos:1024], kT[64:128, ks], qT[64:128, qv],
                    start=True, stop=True,
                )
                ex = expp.tile([128, 1024], fp16, tag="ex", bufs=8, name="ex")
                if pos == 0:
                    nc.scalar.activation(ex, sp, EXP, scale=float(SCALE))
                else:
                    exv = ex.rearrange("p (i n) -> p i n", i=2)[:, :, pos:512]
                    spv = sp.rearrange("p (i n) -> p i n", i=2)[:, :, pos:512]
                    nc.scalar.activation(exv, spv, EXP, scale=float(SCALE))
                if kt * 128 >= qc * 512:  # diagonal tile: mask both heads at once
                    exd = ex.rearrange("p (i n) -> p i n", i=2)[:, :, pos:pos + 128]
                    mkd = msk_t.rearrange("p (i n) -> p i n", i=2)
                    nc.vector.tensor_mul(exd, exd, mkd)
                exs[(hp, qc, kt)] = (ex, pos)

            def A(hp, qc, kt, nkt):
                """attv pair for exp tile (hp, qc, kt)."""
                if (hp, qc) not in yps:
                    yps[(hp, qc)] = (
                        psm.tile([128, 512], f32, tag="yp", name="ypA"),
                        psm.tile([128, 512], f32, tag="yp", name="ypB"),
                    )
                ypA, ypB = yps[(hp, qc)]
                ex, pos = exs.pop((hp, qc, kt))
                for yp, half in ((ypA, 0), (ypB, 1)):
                    nc.tensor.matmul(
                        yp[:DA, pos:512],
                        v_t[kt][:, hp * HB + half * DA:hp * HB + (half + 1) * DA],
                        ex[:, half * 512 + pos:(half + 1) * 512],
                        start=(kt == 0), stop=(kt == nkt - 1),
                    )

            stages = {}
            bcast = {}

            def FIN_sums(hp, qc, tail=False):
                """Issued right after the last attv of the block: pull the
                softmax sums out of PSUM row 64 (DVE copy — DMA cannot read
                PSUM), drop them onto partition 0 of s64r with one gpsimd
                SBUF-to-SBUF DMA, and broadcast them to all partitions.  This
                jumps the vector queue ahead of the block-tail casts so the
                broadcast is long done when FIN_normB consumes it."""
                with nc.allow_low_precision(reason="sums rounding is benign"):
                    for r, yp in enumerate(yps[(hp, qc)]):
                        nc.vector.tensor_copy(
                            s64v[64:65, r * 512:(r + 1) * 512], yp[D:DA, :]
                        )
                if tail:
                    return
                nc.gpsimd.dma_start(out=s64r[0:1, :], in_=s64v[64:65, :])
                bcS = expp.tile([128, 1024], f32, tag="bcS", bufs=2, name="bcS")
                nc.gpsimd.partition_broadcast(bcS, s64r[0:1, :], channels=128)
                bcast[(hp, qc)] = bcS
                # stage the unnormalized y rows right away: the copies drain
                # on the vector queue during the block-tail fills, freeing the
                # psum pair before the next block's first attv (WAR) and
                # keeping the next block's mask muls unqueued.
                st = []
                for yp in yps[(hp, qc)]:
                    stage = expp.tile([D, 512], fp16, tag="ystage", bufs=4, name="stage")
                    nc.vector.tensor_copy(stage, yp[:D, :])
                    st.append(stage)
                stages[(hp, qc)] = st
                del yps[(hp, qc)]

            def FIN_normB(hp, qc):
                """Reciprocal of the broadcast sums, normalize the staged y
                tiles in place, DMA them into yT.  Deferred to the middle of
                the next block so the in-order vector queue never waits on
                the gpsimd broadcast."""
                bcS = bcast.pop((hp, qc))
                bcR = expp.tile([128, 1024], f32, tag="bcR", bufs=2, name="bcR")
                nc.vector.reciprocal_approx_fast(bcR, bcS)
                qs = slice(qc * 512, (qc + 1) * 512)
                for r, (stage, off) in enumerate(zip(stages.pop((hp, qc)), (0, 64))):
                    nc.vector.tensor_mul(
                        stage, stage, bcR[0:D, r * 512:(r + 1) * 512]
                    )
                    # sync queue: keeps the in-order gpsimd queue free for the
                    # next block's sums broadcast (convoy breaker)
                    nc.sync.dma_start(out=yT_t[hp][off:off + 64, qs], in_=stage)

            def FIN_tail(hp, qc):
                """Fast finish for the very last block: broadcast the sums
                with two K=1 PE matmuls (the PE is idle in the tail, and this
                skips the long gpsimd broadcast chain), then reciprocal,
                normalize, ship."""
                bc = psc.tile([128, 1024], f32, tag="ps", name="bc")
                nc.tensor.matmul(
                    bc[:, 0:512], onr[64:65, 0:128], s64v[64:65, 0:512],
                    start=True, stop=True,
                )
                nc.tensor.matmul(
                    bc[:, 512:1024], onr[64:65, 128:256], s64v[64:65, 512:1024],
                    start=True, stop=True,
                )
                bcR = expp.tile([128, 1024], f32, tag="bcR", bufs=2, name="bcR")
                nc.vector.reciprocal_approx_fast(bcR, bc)
                qs = slice(qc * 512, (qc + 1) * 512)
                for r, (stage, off) in enumerate(zip(stages.pop((hp, qc)), (0, 64))):
                    nc.vector.tensor_mul(
                        stage, stage, bcR[0:D, r * 512:(r + 1) * 512]
                    )
                    # sync queue: keeps the in-order gpsimd queue free for the
                    # next block's sums broadcast (convoy breaker)
                    nc.sync.dma_start(out=yT_t[hp][off:off + 64, qs], in_=stage)

            def FIN_mini(tau):
                """Sub-tile finish for the last beta block: query tile tau's
                attv columns are final once A(5,1,kt=tau) completes, so
                normalize and ship just those 128 columns, overlapped under
                the remaining attention.  Broadcast via two K=1 PE matmuls,
                reciprocal, fused normalize-copy, DMA to yT."""
                lo = (tau - 4) * 128
                hi = lo + 128
                ypA, ypB = yps[(5, 1)]
                with nc.allow_low_precision(reason="sums rounding is benign"):
                    nc.vector.tensor_copy(s64v[64:65, lo:hi], ypA[D:DA, lo:hi])
                    nc.vector.tensor_copy(
                        s64v[64:65, 512 + lo:512 + hi], ypB[D:DA, lo:hi]
                    )
                bc = psc.tile([128, 1024], f32, tag="ps", name=f"bcm{tau}")
                nc.tensor.matmul(
                    bc[:, 0:128], onr[64:65, 0:128], s64v[64:65, lo:hi],
                    start=True, stop=True,
                )
                nc.tensor.matmul(
                    bc[:, 128:256], onr[64:65, 128:256],
                    s64v[64:65, 512 + lo:512 + hi],
                    start=True, stop=True,
                )
                bcm = expp.tile([128, 256], f32, tag="bcm", bufs=2, name="bcm")
                nc.vector.reciprocal_approx_fast(bcm, bc[:, 0:256])
                for (yp, off, c0), eng in zip(
                    ((ypA, 0, 0), (ypB, 64, 128)), (nc.gpsimd, nc.sync)
                ):
                    stg = expp.tile([D, 128], fp16, tag="ymini", bufs=4, name="ym")
                    nc.vector.tensor_mul(stg, yp[:D, lo:hi], bcm[0:D, c0:c0 + 128])
                    eng.dma_start(
                        out=yT_t[5][off:off + 64, 512 + lo:512 + hi], in_=stg
                    )
                if tau == 7:
                    del yps[(5, 1)]

            # ---------------- schedule ----------------
            # Head: DMA issue costs ~0.65us per descriptor per engine queue, so
            # split between sync (x stream), scalar (wq + constants) and
            # gpsimd (wv/wp).
            nc.vector.memset(onr[64:65, :], 1.0)
            for m in (0, 6):
                wq_fetch(m, nc.scalar, split=True)
            for kk in range(NK):
                nc.sync.dma_start(out=xT_t[kk][:, 0:512], in_=xTd[:, kk, 0:512])
            for m in (1, 7):
                wq_fetch(m, nc.scalar)
            nc.scalar.dma_start(out=msk_t, in_=msk[:, :])
            nc.scalar.dma_start(out=ones_t, in_=onesc[:, :])
            for kk in range(3):
                nc.gpsimd.dma_start(out=xT_t[kk][:, 512:1024], in_=xTd[:, kk, 512:1024])
            for kk in range(3, NK):
                nc.sync.dma_start(out=xT_t[kk][:, 512:1024], in_=xTd[:, kk, 512:1024])
            nc.gpsimd.dma_start(out=wvd, in_=wvs)

            # ---- pre-alpha: the four QK tiles head pairs 0/1 need ----
            for m in (0, 6):
                drain(gen_qk(m))
            wq_fetch(2, nc.scalar)
            wq_fetch(8, nc.scalar)
            for m in (1, 7):
                drain(gen_qk(m))

            # ---- alpha: query half 0; remaining QK and v tiles as filler.
            # hp 0 runs a stretched weave that absorbs the v tiles ----
            alpha_jobs = {
                0: [gen_v(0), gen_v(1), gen_v(2), gen_v(3)],
                1: [gen_qk(2), gen_qk(8)],
                2: [gen_qk(3), gen_qk(9)],
                3: [gen_qk(4), gen_qk(10)],
                4: [gen_qk(5), gen_qk(11)],
                5: [gen_v(4), gen_v(5), gen_v(6)],
            }
            for hp in range(NK):
                # lazily bind jobs (gen_qk reads wq_tiles at first next())
                gens = alpha_jobs[hp]

                def fill():
                    # NB: the final next() of a generator runs its last chunk
                    # and THEN raises StopIteration, so pop-and-return.
                    if gens:
                        try:
                            next(gens[0])
                        except StopIteration:
                            gens.pop(0)

                xf = 1 if hp == 0 else 0  # extra fills while absorbing v tiles
                S(hp, 0, 0)
                S(hp, 0, 1)
                fill()
                fill()
                for _ in range(xf):
                    fill()
                A(hp, 0, 0, 4)
                S(hp, 0, 2)
                fill()
                for _ in range(2 * xf):
                    fill()
                A(hp, 0, 1, 4)
                S(hp, 0, 3)
                fill()
                fill()
                for _ in range(xf):
                    fill()
                A(hp, 0, 2, 4)
                if hp > 1:
                    FIN_normB(hp - 2, 0)
                fill()
                for _ in range(2 * xf):
                    fill()
                A(hp, 0, 3, 4)
                FIN_sums(hp, 0)
                while gens:
                    fill()
                if hp == 0:
                    wq_fetch(3)
                    wq_fetch(9)
                elif hp == 1:
                    wq_fetch(4)
                    wq_fetch(10)
                elif hp == 2:
                    wq_fetch(5)
                    wq_fetch(11)
                elif hp == 3:
                    nc.gpsimd.dma_start(
                        out=wpall.rearrange("p (i n) -> p i n", i=NK),
                        in_=wp.rearrange("i p n -> p i n"),
                    )

            # ---- beta: query half 1 attention + projection filler ----
            beta_jobs = {
                0: [gen_v(7), gen_P(0, 0, 3, "part"), gen_P(1, 0, 3, "part")],
                1: [gen_P(2, 0, 3, "part"), gen_P(3, 0, 3, "part"),
                    gen_P(0, 3, 6, "out")],
                2: [gen_P(1, 3, 6, "out"), gen_P(2, 3, 6, "out")],
                3: [gen_P(3, 3, 6, "out"), gen_P(4, 0, 3, "part"),
                    gen_P(5, 0, 3, "part")],
                4: [gen_P(6, 0, 3, "part"), gen_P(7, 0, 3, "part")],
                5: [gen_noop(2), gen_P(4, 3, 5, "acc"), gen_P(5, 3, 5, "acc"),
                    gen_P(6, 3, 5, "acc"), gen_P(7, 3, 5, "acc")],
            }
            for hp in range(NK):
                gens = beta_jobs[hp]

                def fill():
                    # NB: the final next() of a generator runs its last chunk
                    # and THEN raises StopIteration, so pop-and-return.
                    if gens:
                        try:
                            next(gens[0])
                        except StopIteration:
                            gens.pop(0)

                S(hp, 1, 0)
                S(hp, 1, 1)
                fill()
                fill()
                A(hp, 1, 0, 8)
                if hp == 5:
                    FIN_normB(4, 1)
                elif hp == 3:
                    FIN_normB(2, 1)
                S(hp, 1, 2)
                fill()
                A(hp, 1, 1, 8)
                S(hp, 1, 3)
                fill()
                A(hp, 1, 2, 8)
                if hp == 0:
                    FIN_normB(5, 0)
                elif hp < 5 and hp != 3:
                    FIN_normB(hp - 1, 1)
                S(hp, 1, 4)
                fill()
                A(hp, 1, 3, 8)
                S(hp, 1, 5)
                fill()
                A(hp, 1, 4, 8)
                if hp == 5:
                    FIN_mini(4)
                S(hp, 1, 6)
                fill()
                A(hp, 1, 5, 8)
                if hp == 5:
                    FIN_mini(5)
                    drain(gen_P(4, 5, 6, "out", nc.scalar))
                S(hp, 1, 7)
                fill()
                A(hp, 1, 6, 8)
                if hp == 5:
                    FIN_mini(6)
                    drain(gen_P(5, 5, 6, "out", nc.sync))
                fill()
                A(hp, 1, 7, 8)
                if hp == 5:
                    FIN_mini(7)
                    while gens:
                        fill()
                    drain(gen_P(6, 5, 6, "out", nc.scalar))
                    drain(gen_P(7, 5, 6, "out", nc.sync))
                else:
                    FIN_sums(hp, 1)
                    while gens:
                        fill()

    nc.compile()
    return nc


_nc = None


def _get_nc():
    global _nc
    if _nc is None:
        _nc = build()
    return _nc


def _host_prep(w_attn, w_proj):
    wq = np.ascontiguousarray(
        w_attn[:, :2 * C].reshape(NK, 128, 2 * NK, 128).transpose(2, 1, 0, 3)
    ).astype(np.float16)
    wv_aug = np.zeros((C, H, DA), np.float32)
    wv_aug[:, :, :D] = w_attn[:, 2 * C:].reshape(C, H, D)
    wv = np.ascontiguousarray(wv_aug.reshape(NK, 128, VW)).astype(np.float16)
    wp = np.ascontiguousarray(w_proj.reshape(NK, 128, C)).astype(np.float16)
    tri = np.triu(np.ones((128, 128), np.float32))
    msk = np.concatenate([tri, tri], axis=1).astype(np.float16)
    onesc = np.ones((128, H), np.float16)
    return wq, wv, wp, msk, onesc


def kernel(x, w_attn, w_proj):
    x = np.asarray(x, dtype=np.float32)
    w_attn = np.asarray(w_attn, dtype=np.float32)
    w_proj = np.asarray(w_proj, dtype=np.float32)
    wq, wv, wp, msk, onesc = _host_prep(w_attn, w_proj)
    in_maps = [
        {
            "xT": np.ascontiguousarray(x[b].T).astype(np.float16),
            "wq": wq,
            "wv": wv,
            "wp": wp,
            "msk": msk,
            "onesc": onesc,
        }
        for b in range(B)
    ]
    last_err = None
    for _attempt in range(3):
        try:
            res = run_bass_kernel_spmd(_get_nc(), in_maps, list(range(B)))
            return np.stack([res.results[b]["out"] for b in range(B)], axis=0)
        except Exception as e:  # transient device wedge: retry
            last_err = e
    raise last_err
